# revision 3
# baseline (speedup 1.0000x reference)
"""Causal single-head attention (B=2, S=4096, D=1024) + RoPE on 8 TRN2 cores,
collective version.

Cores 4b+i (b=batch, i=rank 0..3). Rank i owns:
  - kv chunks [8i, 8i+8) (contiguous oct -> AllGather lands in global order)
  - q chunks QCH[i] = {4t+i : t<4} u {4t+3-i : t>=4} (balanced causal mass 132)
Each core: projects K/V for its oct from its x shard, ropes K, AllGathers
K^T|V within its group of 4; projects+ropes Q for its own q rows; then 8
attention slots with a rank-UNIFORM schedule: slot t attends the first
512*(t+1) kv columns, with the last 512-wide tile masked by a per-rank mask
(0 | TRI | -inf per 128-block, built on device from a tiny selector input).
Output: final softmax-normalized rows (bf16), host just scatters them.

Weights are sharded 6-of-24 dc-blocks per core and AllGathered on device.
Dispatch avoids run_bass_kernel_spmd's host-zero upload by creating donated
output buffers on device.
"""

import sys

sys.path.insert(0, "/opt/trn_rl_repo")

import math
from contextlib import ExitStack

import ml_dtypes
import numpy as np

import concourse.bass as bass
import concourse.tile as tile
from concourse import bacc, mybir
from concourse.bass_utils import run_bass_kernel_spmd
from concourse.masks import make_causal_mask, make_identity

BF16 = mybir.dt.bfloat16
F32 = mybir.dt.float32
NPBF16 = ml_dtypes.bfloat16

B, S, D = 2, 4096, 1024
H = D // 2
C = 128
NQC = S // C                  # 32 chunks
NOC = 8                       # own chunks (q and kv) per core
NOR = NOC * C                 # 1024 own rows
SCALE = 1.0 / math.sqrt(D)
NEG = -30000.0
GROUPS = [[0, 1, 2, 3], [4, 5, 6, 7]]

QCH = [sorted([4 * t + i for t in range(4)] + [4 * t + 3 - i for t in range(4, 8)])
       for i in range(4)]

_CACHE = {}


def _build():
    nc = bacc.Bacc("TRN2", target_bir_lowering=False, debug=False,
                   enable_asserts=False, num_devices=8)

    # per-core shards (all contiguous layouts)
    xq_sh = nc.dram_tensor("xq_sh", [C, 8, NOR], BF16, kind="ExternalInput").ap()
    xkv_sh = nc.dram_tensor("xkv_sh", [C, 8, NOR], BF16, kind="ExternalInput").ap()
    # transposed rope tables for own q rows: c 0..3 cos, 4..7 sin
    csq = nc.dram_tensor("csq", [C, 8, NOR], BF16, kind="ExternalInput").ap()
    # natural rope tables for own kv rows: cols 0:H cos, H:D sin
    cskv = nc.dram_tensor("cskv", [NOR, D], BF16, kind="ExternalInput").ap()
    # weight shard: 6 of the flat-24 [wq|wk|wv] dc-blocks
    wsh = nc.dram_tensor("wsh", [C, 6, D], BF16, kind="ExternalInput").ap()
    # mask selectors: [p, pat(2), blk(4), (a,b)]
    sels = nc.dram_tensor("sels", [C, 2, 4, 2], F32, kind="ExternalInput").ap()

    o_fin = nc.dram_tensor("o_fin", [NOC, C, D], BF16, kind="ExternalOutput").ap()

    with tile.TileContext(nc) as tc, ExitStack() as ctx:
        const_p = ctx.enter_context(tc.tile_pool(name="const", bufs=1))
        dram_p = ctx.enter_context(tc.tile_pool(name="dram", bufs=1, space="DRAM"))
        qt_p = ctx.enter_context(tc.tile_pool(name="qt", bufs=1))

        ident = const_p.tile([C, C], BF16)
        make_identity(nc, ident[:])
        tri = const_p.tile([C, C], F32)
        make_causal_mask(nc, tri[:], mask_val=NEG)
        sels_sb = const_p.tile([C, 2, 4, 2], F32)
        nc.sync.dma_start(sels_sb[:], sels)
        masks_sb = const_p.tile([C, 2, 512], F32)
        for k in range(2):
            for c in range(4):
                nc.vector.tensor_scalar(masks_sb[:, k, c * C:(c + 1) * C], tri[:],
                                        sels_sb[:, k, c, 0:1], sels_sb[:, k, c, 1:2],
                                        op0=mybir.AluOpType.mult,
                                        op1=mybir.AluOpType.add)

        qt_sb = qt_p.tile([C, 8, NOR], BF16, tag="qt")

        # DRAM bounce buffers for the collectives
        wb_in = dram_p.tile([C, 6, D], BF16)
        wb_out = dram_p.tile([4, C, 6, D], BF16)
        kvb_in = dram_p.tile([C, 16, NOR], BF16)
        kvb_out = dram_p.tile([4, C, 16, NOR], BF16)

        nc.gpsimd.dma_start(wb_in[:], wsh)
        nc.gpsimd.collective_compute(
            "AllGather", mybir.AluOpType.bypass, replica_groups=GROUPS,
            ins=[wb_in[:].opt()], outs=[wb_out[:].opt()])

        # ---- phase 1: projections (weights + own-x resident) ----
        with tc.tile_pool(name="w", bufs=1) as w_p, \
             tc.tile_pool(name="ph1", bufs=2) as p1_p, \
             tc.tile_pool(name="kvown", bufs=1) as kvown_p, \
             tc.tile_pool(name="mmps", bufs=2, space="PSUM") as mmps_p, \
             tc.tile_pool(name="accps", bufs=2, space="PSUM") as accps_p, \
             tc.tile_pool(name="tpps", bufs=2, space="PSUM") as tpps_p:

            wall = w_p.tile([C, 24, D], BF16, tag="wall")
            for r in range(4):
                nc.sync.dma_start(wall[:, 6 * r:6 * r + 6, :], wb_out[r])

            xkv_sb = w_p.tile([C, 8, NOR], BF16, tag="xkv")
            nc.sync.dma_start(xkv_sb[:], xkv_sh)
            xq_sb = w_p.tile([C, 8, NOR], BF16, tag="xq")
            nc.sync.dma_start(xq_sb[:], xq_sh)
            csq_sb = w_p.tile([C, 8, NOR], BF16, tag="csq")
            nc.sync.dma_start(csq_sb[:], csq)

            kt_own = kvown_p.tile([C, 8, NOR], BF16, tag="kto")
            v_own = kvown_p.tile([C, 8, D], BF16, tag="vo")

            for g in range(NOC):
                rows = slice(g * C, (g + 1) * C)
                cskv_sb = p1_p.tile([C, D], BF16, tag="cskv", name=f"cskv_{g}")
                nc.sync.dma_start(cskv_sb[:], cskv[rows, :])
                k_ps = accps_p.tile([C, D], F32, tag="acc", name=f"kps_{g}")
                v_ps = accps_p.tile([C, D], F32, tag="acc", name=f"vps_{g}")
                for h in range(2):
                    cols = slice(h * 512, (h + 1) * 512)
                    for dc in range(8):
                        nc.tensor.matmul(k_ps[:, cols], xkv_sb[:, dc, rows],
                                         wall[:, 8 + dc, cols],
                                         start=(dc == 0), stop=(dc == 7))
                    for dc in range(8):
                        nc.tensor.matmul(v_ps[:, cols], xkv_sb[:, dc, rows],
                                         wall[:, 16 + dc, cols],
                                         start=(dc == 0), stop=(dc == 7))
                nc.scalar.copy(v_own[:, g, :], v_ps[:])

                kr_sb = p1_p.tile([C, D], BF16, tag="kr", name=f"kr_{g}")
                t0 = p1_p.tile([C, H], BF16, tag="t0", name=f"kt0_{g}")
                t1 = p1_p.tile([C, H], BF16, tag="t1", name=f"kt1_{g}")
                re, im = k_ps[:, 0:H], k_ps[:, H:D]
                ck, sk = cskv_sb[:, 0:H], cskv_sb[:, H:D]
                nc.vector.tensor_mul(t0[:], re, ck)
                nc.vector.tensor_mul(t1[:], im, sk)
                nc.vector.tensor_sub(kr_sb[:, 0:H], t0[:], t1[:])
                nc.vector.tensor_mul(t0[:], re, sk)
                nc.vector.tensor_mul(t1[:], im, ck)
                nc.vector.tensor_add(kr_sb[:, H:D], t0[:], t1[:])

                for dc in range(8):
                    tp = tpps_p.tile([C, 1024], BF16, tag="tp", name=f"ktp_{g}_{dc}")
                    nc.tensor.transpose(tp[:, 0:C], kr_sb[:, dc * C:(dc + 1) * C],
                                        ident[:])
                    nc.scalar.copy(kt_own[:, dc, g * C:(g + 1) * C], tp[:, 0:C])

            # ship own K^T|V to the group
            nc.sync.dma_start(kvb_in[:, 0:8, :], kt_own[:])
            nc.sync.dma_start(kvb_in[:, 8:16, :], v_own[:])
            nc.gpsimd.collective_compute(
                "AllGather", mybir.AluOpType.bypass, replica_groups=GROUPS,
                ins=[kvb_in[:].opt()], outs=[kvb_out[:].opt()])

            # ---- Q projection + rope (overlaps the kv gather) ----
            qraw_sb = w_p.tile([C, 8, NOR], BF16, tag="qraw")
            for gq in range(2):
                qcols = slice(gq * 512, (gq + 1) * 512)
                for e in range(8):
                    qp = mmps_p.tile([C, 512], F32, tag="mm", name=f"qp_{gq}_{e}")
                    for dc in range(8):
                        nc.tensor.matmul(qp[:, 0:512],
                                         wall[:, dc, e * C:(e + 1) * C],
                                         xq_sb[:, dc, qcols],
                                         start=(dc == 0), stop=(dc == 7))
                    nc.scalar.copy(qraw_sb[:, e, qcols], qp[:, 0:512])
            for ec in range(4):
                cc, ss = csq_sb[:, ec, :], csq_sb[:, ec + 4, :]
                re, im = qraw_sb[:, ec, :], qraw_sb[:, ec + 4, :]
                t0 = p1_p.tile([C, NOR], BF16, tag="rt0", name=f"rt0_{ec}")
                t1 = p1_p.tile([C, NOR], BF16, tag="rt1", name=f"rt1_{ec}")
                nc.vector.tensor_mul(t0[:], re, cc)
                nc.vector.tensor_mul(t1[:], im, ss)
                nc.vector.tensor_sub(qt_sb[:, ec, :], t0[:], t1[:])
                t2 = p1_p.tile([C, NOR], BF16, tag="rt2", name=f"rt2_{ec}")
                t3 = p1_p.tile([C, NOR], BF16, tag="rt3", name=f"rt3_{ec}")
                nc.vector.tensor_mul(t2[:], re, ss)
                nc.vector.tensor_mul(t3[:], im, cc)
                nc.vector.tensor_add(qt_sb[:, ec + 4, :], t2[:], t3[:])

        # ---- phase 2: attention on gathered kv ----
        kvall_p = ctx.enter_context(tc.tile_pool(name="kvall", bufs=1))
        kt_all = kvall_p.tile([C, 4, 8, NOR], BF16, tag="kta")   # [p, r, dc, pos]
        v_all = kvall_p.tile([C, 4, 8, D], BF16, tag="va")       # [p, r, chunk, d]
        for r in range(4):
            nc.sync.dma_start(kt_all[:, r, :, :], kvb_out[r, :, 0:8, :])
            nc.sync.dma_start(v_all[:, r, :, :], kvb_out[r, :, 8:16, :])

        with tc.tile_pool(name="sc", bufs=1) as sc_p, \
             tc.tile_pool(name="at", bufs=2) as at_p, \
             tc.tile_pool(name="mmps2", bufs=2, space="PSUM") as mmps2_p, \
             tc.tile_pool(name="accps2", bufs=2, space="PSUM") as accps2_p, \
             tc.tile_pool(name="tpps2", bufs=2, space="PSUM") as tpps2_p:

            for t in range(NOC):
                W = 512 * (t + 1)
                kpat = 0 if t < 4 else 1
                qc = slice(t * C, (t + 1) * C)
                sc_sb = sc_p.tile([C, S], F32, tag="scores", name=f"sc_{t}")
                for u in range(t + 1):
                    cols = slice(u * 512, (u + 1) * 512)
                    s_ps = mmps2_p.tile([C, 512], F32, tag="mm", name=f"sps_{t}_{u}")
                    for dc in range(8):
                        nc.tensor.matmul(s_ps[:, 0:512], qt_sb[:, dc, qc],
                                         kt_all[:, u // 2, dc,
                                                (u % 2) * 512:(u % 2) * 512 + 512],
                                         start=(dc == 0), stop=(dc == 7))
                    if u == t:
                        nc.vector.tensor_add(sc_sb[:, cols], s_ps[:, 0:512],
                                             masks_sb[:, kpat, :])
                    else:
                        nc.scalar.copy(sc_sb[:, cols], s_ps[:, 0:512])

                rmax = at_p.tile([C, 1], F32, tag="rmax", name=f"rmax_{t}")
                nc.vector.tensor_reduce(rmax[:], sc_sb[:, 0:W],
                                        axis=mybir.AxisListType.X,
                                        op=mybir.AluOpType.max)
                negm = at_p.tile([C, 1], F32, tag="negm", name=f"negm_{t}")
                nc.scalar.mul(negm[:], rmax[:], -SCALE)
                p_sb = at_p.tile([C, S], BF16, tag="p", name=f"p_{t}")
                lsum = at_p.tile([C, 1], F32, tag="lsum", name=f"lsum_{t}")
                nc.scalar.activation(p_sb[:, 0:W], sc_sb[:, 0:W],
                                     mybir.ActivationFunctionType.Exp,
                                     bias=negm[:], scale=SCALE,
                                     accum_out=lsum[:])
                rinv = at_p.tile([C, 1], F32, tag="rinv", name=f"rinv_{t}")
                nc.vector.reciprocal(rinv[:], lsum[:])

                o_ps = accps2_p.tile([C, D], F32, tag="acc", name=f"ops_{t}")
                nsub = W // C
                for s0 in range(0, nsub, 2):
                    npair = min(2, nsub - s0)
                    ptp = tpps2_p.tile([C, 1024], BF16, tag="tp", name=f"ptp_{t}_{s0}")
                    for u in range(npair):
                        nc.tensor.transpose(ptp[:, u * C:(u + 1) * C],
                                            p_sb[:, (s0 + u) * C:(s0 + u + 1) * C],
                                            ident[:])
                    pt_sb = at_p.tile([C, 2 * C], BF16, tag="pt", name=f"pt_{t}_{s0}")
                    nc.scalar.copy(pt_sb[:, 0:npair * C], ptp[:, 0:npair * C])
                    for u in range(npair):
                        sI = s0 + u
                        for h in range(2):
                            cols = slice(h * 512, (h + 1) * 512)
                            nc.tensor.matmul(o_ps[:, cols],
                                             pt_sb[:, u * C:(u + 1) * C],
                                             v_all[:, sI // 8, sI % 8, cols],
                                             start=(sI == 0), stop=(sI == nsub - 1))
                ob_sb = at_p.tile([C, D], BF16, tag="ob", name=f"ob_{t}")
                nc.vector.tensor_scalar_mul(ob_sb[:], o_ps[:], rinv[:])
                nc.sync.dma_start(o_fin[t], ob_sb[:])

    nc.compile()
    return nc


def _prep_inputs(x, w_q, w_k, w_v, freqs_cos, freqs_sin):
    perm = np.concatenate([np.arange(0, D, 2), np.arange(1, D, 2)])
    wqT = np.ascontiguousarray(w_q[perm, :].T.astype(NPBF16))
    wkT = np.ascontiguousarray(w_k[perm, :].T.astype(NPBF16))
    wvT = np.ascontiguousarray(w_v.T.astype(NPBF16))

    def blk(wt):  # [D, D] -> [p, dc, e]
        return wt.reshape(8, C, D).transpose(1, 0, 2)

    flat24 = np.ascontiguousarray(
        np.concatenate([blk(wqT), blk(wkT), blk(wvT)], axis=1))  # [C, 24, D]
    cosb = freqs_cos.astype(NPBF16)
    sinb = freqs_sin.astype(NPBF16)

    def xt_blocked(rows_x):  # [n, D] -> [p, dc, n]
        return np.ascontiguousarray(rows_x.reshape(-1, 8, C).transpose(2, 1, 0))

    in_maps = []
    for core in range(8):
        b, i = divmod(core, 4)
        xb = np.asarray(x[b]).astype(NPBF16)
        qrows = (np.asarray(QCH[i])[:, None] * C + np.arange(C)[None, :]).reshape(-1)
        kvrows = np.arange(i * NOR, (i + 1) * NOR)
        csq_n = np.concatenate([cosb[qrows], sinb[qrows]], axis=1)  # [NOR, D]
        in_maps.append({
            "xq_sh": xt_blocked(xb[qrows]),
            "xkv_sh": xt_blocked(xb[kvrows]),
            "csq": xt_blocked(csq_n),
            "cskv": np.ascontiguousarray(
                np.concatenate([cosb[kvrows], sinb[kvrows]], axis=1)),
            "wsh": np.ascontiguousarray(flat24[:, 6 * i:6 * i + 6, :]),
            "sels": _sels(i),
        })
    return in_maps


def _sels(i):
    s = np.zeros((2, 4, 2), np.float32)
    for k, diag in enumerate((i, 3 - i)):
        for c in range(4):
            s[k, c, 0] = 1.0 if c == diag else 0.0
            s[k, c, 1] = NEG if c > diag else 0.0
    return np.ascontiguousarray(np.broadcast_to(s[None], (C, 2, 4, 2)))


def _assemble(results):
    out = np.empty((B, S, D), np.float32)
    for core in range(8):
        b, i = divmod(core, 4)
        o = np.asarray(results[core]["o_fin"], np.float32)  # [NOC, C, D]
        for t, j in enumerate(QCH[i]):
            out[b, j * C:(j + 1) * C] = o[t]
    return out


def _run_pjrt(nc, in_maps, n_cores=8):
    """Like bass2jax.run_bass_via_pjrt, but creates the donated output
    buffers ON DEVICE (jit zeros) instead of uploading host zeros."""
    import jax
    import jax.numpy as jnp
    from jax.sharding import Mesh, NamedSharding, PartitionSpec
    try:
        from jax import shard_map
    except ImportError:
        from jax.experimental.shard_map import shard_map
    from concourse.bass2jax import (_bass_exec_p, install_neuronx_cc_hook,
                                    partition_id_tensor)

    install_neuronx_cc_hook()
    partition_name = nc.partition_id_tensor.name if nc.partition_id_tensor else None
    in_names, out_names, out_avals = [], [], []
    for alloc in nc.m.functions[0].allocations:
        if not isinstance(alloc, mybir.MemoryLocationSet):
            continue
        name = alloc.memorylocations[0].name
        if alloc.kind == "ExternalInput":
            if name != partition_name:
                in_names.append(name)
        elif alloc.kind == "ExternalOutput":
            out_avals.append(jax.core.ShapedArray(
                tuple(alloc.tensor_shape), mybir.dt.np(alloc.dtype)))
            out_names.append(name)
    n_params = len(in_names)
    all_in = list(in_names) + list(out_names)
    if partition_name is not None:
        all_in.append(partition_name)
    donate = tuple(range(n_params, n_params + len(out_names)))

    def _body(*args):
        operands = list(args)
        if partition_name is not None:
            operands.append(partition_id_tensor())
        return tuple(_bass_exec_p.bind(
            *operands, out_avals=tuple(out_avals), in_names=tuple(all_in),
            out_names=tuple(out_names), lowering_input_output_aliases=(),
            sim_require_finite=True, sim_require_nnan=True, nc=nc))

    devices = jax.devices()[:n_cores]
    mesh = Mesh(np.asarray(devices), ("core",))
    nio = n_params + len(out_names)
    sm_kw = dict(mesh=mesh, in_specs=(PartitionSpec("core"),) * nio,
                 out_specs=(PartitionSpec("core"),) * len(out_names))
    try:
        smapped = shard_map(_body, check_vma=False, **sm_kw)
    except TypeError:
        smapped = shard_map(_body, check_rep=False, **sm_kw)
    sharded = jax.jit(smapped, donate_argnums=donate, keep_unused=True)
    sh = NamedSharding(mesh, PartitionSpec("core"))
    zeros = jax.jit(
        lambda: tuple(jnp.zeros((n_cores * a.shape[0], *a.shape[1:]), a.dtype)
                      for a in out_avals),
        out_shardings=(sh,) * len(out_avals))()
    concat_in = [np.concatenate([np.asarray(m[k]) for m in in_maps], axis=0)
                 for k in in_names]
    outs = [np.asarray(o) for o in sharded(*concat_in, *zeros)]
    per_core = []
    for c in range(n_cores):
        d = {}
        for name, arr in zip(out_names, outs):
            s0 = arr.shape[0] // n_cores
            d[name] = arr[c * s0:(c + 1) * s0]
        per_core.append(d)
    return per_core


def kernel(x, w_q, w_k, w_v, freqs_cos, freqs_sin, _want_results=False, _trace=False):
    if "nc" not in _CACHE:
        _CACHE["nc"] = _build()
    nc = _CACHE["nc"]
    in_maps = _prep_inputs(np.asarray(x, np.float32), np.asarray(w_q, np.float32),
                           np.asarray(w_k, np.float32), np.asarray(w_v, np.float32),
                           np.asarray(freqs_cos, np.float32),
                           np.asarray(freqs_sin, np.float32))
    if _trace:
        kr = run_bass_kernel_spmd(nc, in_maps, core_ids=list(range(8)), trace=True)
        out = _assemble(kr.results)
        return (out, kr) if _want_results else out
    try:
        results = _run_pjrt(nc, in_maps)
    except Exception as e:
        print(f"kernel: _run_pjrt failed ({type(e).__name__}: {e}); "
              "falling back to run_bass_kernel_spmd", file=sys.stderr)
        kr = run_bass_kernel_spmd(nc, in_maps, core_ids=list(range(8)))
        results = kr.results
    out = _assemble(results)
    if _want_results:
        return out, results
    return out


# revision 4
# speedup vs baseline: 1.0079x; 1.0079x over previous
"""Causal single-head attention (B=2, S=4096, D=1024) + RoPE on 8 TRN2 cores,
collective version.

Cores 4b+i (b=batch, i=rank 0..3). Rank i owns:
  - kv chunks [8i, 8i+8) (contiguous oct -> AllGather lands in global order)
  - q chunks QCH[i] = {4t+i : t<4} u {4t+3-i : t>=4} (balanced causal mass 132)
Each core: projects K/V for its oct from its x shard, ropes K, AllGathers
K^T|V within its group of 4; projects+ropes Q for its own q rows; then 8
attention slots with a rank-UNIFORM schedule: slot t attends the first
512*(t+1) kv columns, with the last 512-wide tile masked by a per-rank mask
(0 | TRI | -inf per 128-block, built on device from a tiny selector input).
Output: final softmax-normalized rows (bf16), host just scatters them.

Weights are sharded 3-of-24 dc-blocks per core and AllGathered across all 8
cores (both batch groups share the same weights).
Dispatch avoids run_bass_kernel_spmd's host-zero upload by creating donated
output buffers on device.
"""

import sys

sys.path.insert(0, "/opt/trn_rl_repo")

import math
from contextlib import ExitStack

import ml_dtypes
import numpy as np

import concourse.bass as bass
import concourse.tile as tile
from concourse import bacc, mybir
from concourse.bass_utils import run_bass_kernel_spmd
from concourse.masks import make_causal_mask, make_identity

BF16 = mybir.dt.bfloat16
F32 = mybir.dt.float32
NPBF16 = ml_dtypes.bfloat16

B, S, D = 2, 4096, 1024
H = D // 2
C = 128
NQC = S // C                  # 32 chunks
NOC = 8                       # own chunks (q and kv) per core
NOR = NOC * C                 # 1024 own rows
SCALE = 1.0 / math.sqrt(D)
NEG = -30000.0
GROUPS = [[0, 1, 2, 3], [4, 5, 6, 7]]

QCH = [sorted([4 * t + i for t in range(4)] + [4 * t + 3 - i for t in range(4, 8)])
       for i in range(4)]

_CACHE = {}


def _build():
    nc = bacc.Bacc("TRN2", target_bir_lowering=False, debug=False,
                   enable_asserts=False, num_devices=8)

    # per-core shards (all contiguous layouts)
    xq_sh = nc.dram_tensor("xq_sh", [C, 8, NOR], BF16, kind="ExternalInput").ap()
    xkv_sh = nc.dram_tensor("xkv_sh", [C, 8, NOR], BF16, kind="ExternalInput").ap()
    # transposed rope tables for own q rows: c 0..3 cos, 4..7 sin
    csq = nc.dram_tensor("csq", [C, 8, NOR], BF16, kind="ExternalInput").ap()
    # natural rope tables for own kv rows: cols 0:H cos, H:D sin
    cskv = nc.dram_tensor("cskv", [NOR, D], BF16, kind="ExternalInput").ap()
    # weight shard: 3 of the flat-24 [wq|wk|wv] dc-blocks (8-rank gather)
    wsh = nc.dram_tensor("wsh", [C, 3, D], BF16, kind="ExternalInput").ap()
    # mask selectors: [p, pat(2), blk(4), (a,b)]
    sels = nc.dram_tensor("sels", [C, 2, 4, 2], F32, kind="ExternalInput").ap()

    o_fin = nc.dram_tensor("o_fin", [NOC, C, D], BF16, kind="ExternalOutput").ap()

    with tile.TileContext(nc) as tc, ExitStack() as ctx:
        const_p = ctx.enter_context(tc.tile_pool(name="const", bufs=1))
        dram_p = ctx.enter_context(tc.tile_pool(name="dram", bufs=1, space="DRAM"))
        qt_p = ctx.enter_context(tc.tile_pool(name="qt", bufs=1))

        ident = const_p.tile([C, C], BF16)
        make_identity(nc, ident[:])
        tri = const_p.tile([C, C], F32)
        make_causal_mask(nc, tri[:], mask_val=NEG)
        sels_sb = const_p.tile([C, 2, 4, 2], F32)
        nc.sync.dma_start(sels_sb[:], sels)
        masks_sb = const_p.tile([C, 2, 512], F32)
        for k in range(2):
            for c in range(4):
                nc.vector.tensor_scalar(masks_sb[:, k, c * C:(c + 1) * C], tri[:],
                                        sels_sb[:, k, c, 0:1], sels_sb[:, k, c, 1:2],
                                        op0=mybir.AluOpType.mult,
                                        op1=mybir.AluOpType.add)

        qt_sb = qt_p.tile([C, 8, NOR], BF16, tag="qt")

        # DRAM bounce buffers for the collectives
        wb_in = dram_p.tile([C, 3, D], BF16)
        wb_out = dram_p.tile([8, C, 3, D], BF16)
        kvb_in = dram_p.tile([C, 16, NOR], BF16)
        kvb_out = dram_p.tile([4, C, 16, NOR], BF16)

        nc.gpsimd.dma_start(wb_in[:], wsh)
        nc.gpsimd.collective_compute(
            "AllGather", mybir.AluOpType.bypass, replica_groups=[list(range(8))],
            ins=[wb_in[:].opt()], outs=[wb_out[:].opt()])

        # ---- phase 1: projections (weights + own-x resident) ----
        with tc.tile_pool(name="w", bufs=1) as w_p, \
             tc.tile_pool(name="ph1", bufs=2) as p1_p, \
             tc.tile_pool(name="kvown", bufs=1) as kvown_p, \
             tc.tile_pool(name="mmps", bufs=2, space="PSUM") as mmps_p, \
             tc.tile_pool(name="accps", bufs=2, space="PSUM") as accps_p, \
             tc.tile_pool(name="tpps", bufs=2, space="PSUM") as tpps_p:

            wall = w_p.tile([C, 24, D], BF16, tag="wall")
            for r in range(8):
                nc.sync.dma_start(wall[:, 3 * r:3 * r + 3, :], wb_out[r])

            xkv_sb = w_p.tile([C, 8, NOR], BF16, tag="xkv")
            nc.sync.dma_start(xkv_sb[:], xkv_sh)
            xq_sb = w_p.tile([C, 8, NOR], BF16, tag="xq")
            nc.sync.dma_start(xq_sb[:], xq_sh)
            csq_sb = w_p.tile([C, 8, NOR], BF16, tag="csq")
            nc.sync.dma_start(csq_sb[:], csq)

            kt_own = kvown_p.tile([C, 8, NOR], BF16, tag="kto")
            v_own = kvown_p.tile([C, 8, D], BF16, tag="vo")

            for g in range(NOC):
                rows = slice(g * C, (g + 1) * C)
                cskv_sb = p1_p.tile([C, D], BF16, tag="cskv", name=f"cskv_{g}")
                nc.sync.dma_start(cskv_sb[:], cskv[rows, :])
                k_ps = accps_p.tile([C, D], F32, tag="acc", name=f"kps_{g}")
                v_ps = accps_p.tile([C, D], F32, tag="acc", name=f"vps_{g}")
                for h in range(2):
                    cols = slice(h * 512, (h + 1) * 512)
                    for dc in range(8):
                        nc.tensor.matmul(k_ps[:, cols], xkv_sb[:, dc, rows],
                                         wall[:, 8 + dc, cols],
                                         start=(dc == 0), stop=(dc == 7))
                    for dc in range(8):
                        nc.tensor.matmul(v_ps[:, cols], xkv_sb[:, dc, rows],
                                         wall[:, 16 + dc, cols],
                                         start=(dc == 0), stop=(dc == 7))
                nc.scalar.copy(v_own[:, g, :], v_ps[:])

                kr_sb = p1_p.tile([C, D], BF16, tag="kr", name=f"kr_{g}")
                t0 = p1_p.tile([C, H], BF16, tag="t0", name=f"kt0_{g}")
                t1 = p1_p.tile([C, H], BF16, tag="t1", name=f"kt1_{g}")
                re, im = k_ps[:, 0:H], k_ps[:, H:D]
                ck, sk = cskv_sb[:, 0:H], cskv_sb[:, H:D]
                nc.vector.tensor_mul(t0[:], re, ck)
                nc.vector.tensor_mul(t1[:], im, sk)
                nc.vector.tensor_sub(kr_sb[:, 0:H], t0[:], t1[:])
                nc.vector.tensor_mul(t0[:], re, sk)
                nc.vector.tensor_mul(t1[:], im, ck)
                nc.vector.tensor_add(kr_sb[:, H:D], t0[:], t1[:])

                for dc in range(8):
                    tp = tpps_p.tile([C, 1024], BF16, tag="tp", name=f"ktp_{g}_{dc}")
                    nc.tensor.transpose(tp[:, 0:C], kr_sb[:, dc * C:(dc + 1) * C],
                                        ident[:])
                    nc.scalar.copy(kt_own[:, dc, g * C:(g + 1) * C], tp[:, 0:C])

            # ship own K^T|V to the group
            nc.sync.dma_start(kvb_in[:, 0:8, :], kt_own[:])
            nc.sync.dma_start(kvb_in[:, 8:16, :], v_own[:])
            nc.gpsimd.collective_compute(
                "AllGather", mybir.AluOpType.bypass, replica_groups=GROUPS,
                ins=[kvb_in[:].opt()], outs=[kvb_out[:].opt()])

            # ---- Q projection + rope (overlaps the kv gather) ----
            qraw_sb = w_p.tile([C, 8, NOR], BF16, tag="qraw")
            for gq in range(2):
                qcols = slice(gq * 512, (gq + 1) * 512)
                for e in range(8):
                    qp = mmps_p.tile([C, 512], F32, tag="mm", name=f"qp_{gq}_{e}")
                    for dc in range(8):
                        nc.tensor.matmul(qp[:, 0:512],
                                         wall[:, dc, e * C:(e + 1) * C],
                                         xq_sb[:, dc, qcols],
                                         start=(dc == 0), stop=(dc == 7))
                    nc.scalar.copy(qraw_sb[:, e, qcols], qp[:, 0:512])
            for ec in range(4):
                cc, ss = csq_sb[:, ec, :], csq_sb[:, ec + 4, :]
                re, im = qraw_sb[:, ec, :], qraw_sb[:, ec + 4, :]
                t0 = p1_p.tile([C, NOR], BF16, tag="rt0", name=f"rt0_{ec}")
                t1 = p1_p.tile([C, NOR], BF16, tag="rt1", name=f"rt1_{ec}")
                nc.vector.tensor_mul(t0[:], re, cc)
                nc.vector.tensor_mul(t1[:], im, ss)
                nc.vector.tensor_sub(qt_sb[:, ec, :], t0[:], t1[:])
                t2 = p1_p.tile([C, NOR], BF16, tag="rt2", name=f"rt2_{ec}")
                t3 = p1_p.tile([C, NOR], BF16, tag="rt3", name=f"rt3_{ec}")
                nc.vector.tensor_mul(t2[:], re, ss)
                nc.vector.tensor_mul(t3[:], im, cc)
                nc.vector.tensor_add(qt_sb[:, ec + 4, :], t2[:], t3[:])

        # ---- phase 2: attention on gathered kv ----
        kvall_p = ctx.enter_context(tc.tile_pool(name="kvall", bufs=1))
        kt_all = kvall_p.tile([C, 4, 8, NOR], BF16, tag="kta")   # [p, r, dc, pos]
        v_all = kvall_p.tile([C, 4, 8, D], BF16, tag="va")       # [p, r, chunk, d]
        for r in range(4):
            nc.sync.dma_start(kt_all[:, r, :, :], kvb_out[r, :, 0:8, :])
            nc.sync.dma_start(v_all[:, r, :, :], kvb_out[r, :, 8:16, :])

        with tc.tile_pool(name="sc", bufs=1) as sc_p, \
             tc.tile_pool(name="at", bufs=2) as at_p, \
             tc.tile_pool(name="mmps2", bufs=2, space="PSUM") as mmps2_p, \
             tc.tile_pool(name="accps2", bufs=2, space="PSUM") as accps2_p, \
             tc.tile_pool(name="tpps2", bufs=2, space="PSUM") as tpps2_p:

            for t in range(NOC):
                W = 512 * (t + 1)
                kpat = 0 if t < 4 else 1
                qc = slice(t * C, (t + 1) * C)
                sc_sb = sc_p.tile([C, S], F32, tag="scores", name=f"sc_{t}")
                for u in range(t + 1):
                    cols = slice(u * 512, (u + 1) * 512)
                    s_ps = mmps2_p.tile([C, 512], F32, tag="mm", name=f"sps_{t}_{u}")
                    for dc in range(8):
                        nc.tensor.matmul(s_ps[:, 0:512], qt_sb[:, dc, qc],
                                         kt_all[:, u // 2, dc,
                                                (u % 2) * 512:(u % 2) * 512 + 512],
                                         start=(dc == 0), stop=(dc == 7))
                    if u == t:
                        nc.vector.tensor_add(sc_sb[:, cols], s_ps[:, 0:512],
                                             masks_sb[:, kpat, :])
                    else:
                        nc.scalar.copy(sc_sb[:, cols], s_ps[:, 0:512])

                rmax = at_p.tile([C, 1], F32, tag="rmax", name=f"rmax_{t}")
                nc.vector.tensor_reduce(rmax[:], sc_sb[:, 0:W],
                                        axis=mybir.AxisListType.X,
                                        op=mybir.AluOpType.max)
                negm = at_p.tile([C, 1], F32, tag="negm", name=f"negm_{t}")
                nc.scalar.mul(negm[:], rmax[:], -SCALE)
                p_sb = at_p.tile([C, S], BF16, tag="p", name=f"p_{t}")
                lsum = at_p.tile([C, 1], F32, tag="lsum", name=f"lsum_{t}")
                nc.scalar.activation(p_sb[:, 0:W], sc_sb[:, 0:W],
                                     mybir.ActivationFunctionType.Exp,
                                     bias=negm[:], scale=SCALE,
                                     accum_out=lsum[:])
                rinv = at_p.tile([C, 1], F32, tag="rinv", name=f"rinv_{t}")
                nc.vector.reciprocal(rinv[:], lsum[:])

                o_ps = accps2_p.tile([C, D], F32, tag="acc", name=f"ops_{t}")
                nsub = W // C
                for s0 in range(0, nsub, 2):
                    npair = min(2, nsub - s0)
                    ptp = tpps2_p.tile([C, 1024], BF16, tag="tp", name=f"ptp_{t}_{s0}")
                    for u in range(npair):
                        nc.tensor.transpose(ptp[:, u * C:(u + 1) * C],
                                            p_sb[:, (s0 + u) * C:(s0 + u + 1) * C],
                                            ident[:])
                    pt_sb = at_p.tile([C, 2 * C], BF16, tag="pt", name=f"pt_{t}_{s0}")
                    nc.scalar.copy(pt_sb[:, 0:npair * C], ptp[:, 0:npair * C])
                    for u in range(npair):
                        sI = s0 + u
                        for h in range(2):
                            cols = slice(h * 512, (h + 1) * 512)
                            nc.tensor.matmul(o_ps[:, cols],
                                             pt_sb[:, u * C:(u + 1) * C],
                                             v_all[:, sI // 8, sI % 8, cols],
                                             start=(sI == 0), stop=(sI == nsub - 1))
                ob_sb = at_p.tile([C, D], BF16, tag="ob", name=f"ob_{t}")
                nc.vector.tensor_scalar_mul(ob_sb[:], o_ps[:], rinv[:])
                nc.sync.dma_start(o_fin[t], ob_sb[:])

    nc.compile()
    return nc


def _prep_inputs(x, w_q, w_k, w_v, freqs_cos, freqs_sin):
    perm = np.concatenate([np.arange(0, D, 2), np.arange(1, D, 2)])
    wqT = np.ascontiguousarray(w_q[perm, :].T.astype(NPBF16))
    wkT = np.ascontiguousarray(w_k[perm, :].T.astype(NPBF16))
    wvT = np.ascontiguousarray(w_v.T.astype(NPBF16))

    def blk(wt):  # [D, D] -> [p, dc, e]
        return wt.reshape(8, C, D).transpose(1, 0, 2)

    flat24 = np.ascontiguousarray(
        np.concatenate([blk(wqT), blk(wkT), blk(wvT)], axis=1))  # [C, 24, D]
    cosb = freqs_cos.astype(NPBF16)
    sinb = freqs_sin.astype(NPBF16)

    def xt_blocked(rows_x):  # [n, D] -> [p, dc, n]
        return np.ascontiguousarray(rows_x.reshape(-1, 8, C).transpose(2, 1, 0))

    in_maps = []
    for core in range(8):
        b, i = divmod(core, 4)
        xb = np.asarray(x[b]).astype(NPBF16)
        qrows = (np.asarray(QCH[i])[:, None] * C + np.arange(C)[None, :]).reshape(-1)
        kvrows = np.arange(i * NOR, (i + 1) * NOR)
        csq_n = np.concatenate([cosb[qrows], sinb[qrows]], axis=1)  # [NOR, D]
        in_maps.append({
            "xq_sh": xt_blocked(xb[qrows]),
            "xkv_sh": xt_blocked(xb[kvrows]),
            "csq": xt_blocked(csq_n),
            "cskv": np.ascontiguousarray(
                np.concatenate([cosb[kvrows], sinb[kvrows]], axis=1)),
            "wsh": np.ascontiguousarray(
                flat24[:, 3 * (4 * b + i):3 * (4 * b + i) + 3, :]),
            "sels": _sels(i),
        })
    return in_maps


def _sels(i):
    s = np.zeros((2, 4, 2), np.float32)
    for k, diag in enumerate((i, 3 - i)):
        for c in range(4):
            s[k, c, 0] = 1.0 if c == diag else 0.0
            s[k, c, 1] = NEG if c > diag else 0.0
    return np.ascontiguousarray(np.broadcast_to(s[None], (C, 2, 4, 2)))


def _assemble(results):
    out = np.empty((B, S, D), np.float32)
    for core in range(8):
        b, i = divmod(core, 4)
        o = np.asarray(results[core]["o_fin"], np.float32)  # [NOC, C, D]
        for t, j in enumerate(QCH[i]):
            out[b, j * C:(j + 1) * C] = o[t]
    return out


def _run_pjrt(nc, in_maps, n_cores=8):
    """Like bass2jax.run_bass_via_pjrt, but creates the donated output
    buffers ON DEVICE (jit zeros) instead of uploading host zeros."""
    import jax
    import jax.numpy as jnp
    from jax.sharding import Mesh, NamedSharding, PartitionSpec
    try:
        from jax import shard_map
    except ImportError:
        from jax.experimental.shard_map import shard_map
    from concourse.bass2jax import (_bass_exec_p, install_neuronx_cc_hook,
                                    partition_id_tensor)

    install_neuronx_cc_hook()
    partition_name = nc.partition_id_tensor.name if nc.partition_id_tensor else None
    in_names, out_names, out_avals = [], [], []
    for alloc in nc.m.functions[0].allocations:
        if not isinstance(alloc, mybir.MemoryLocationSet):
            continue
        name = alloc.memorylocations[0].name
        if alloc.kind == "ExternalInput":
            if name != partition_name:
                in_names.append(name)
        elif alloc.kind == "ExternalOutput":
            out_avals.append(jax.core.ShapedArray(
                tuple(alloc.tensor_shape), mybir.dt.np(alloc.dtype)))
            out_names.append(name)
    n_params = len(in_names)
    all_in = list(in_names) + list(out_names)
    if partition_name is not None:
        all_in.append(partition_name)
    donate = tuple(range(n_params, n_params + len(out_names)))

    def _body(*args):
        operands = list(args)
        if partition_name is not None:
            operands.append(partition_id_tensor())
        return tuple(_bass_exec_p.bind(
            *operands, out_avals=tuple(out_avals), in_names=tuple(all_in),
            out_names=tuple(out_names), lowering_input_output_aliases=(),
            sim_require_finite=True, sim_require_nnan=True, nc=nc))

    devices = jax.devices()[:n_cores]
    mesh = Mesh(np.asarray(devices), ("core",))
    nio = n_params + len(out_names)
    sm_kw = dict(mesh=mesh, in_specs=(PartitionSpec("core"),) * nio,
                 out_specs=(PartitionSpec("core"),) * len(out_names))
    try:
        smapped = shard_map(_body, check_vma=False, **sm_kw)
    except TypeError:
        smapped = shard_map(_body, check_rep=False, **sm_kw)
    sharded = jax.jit(smapped, donate_argnums=donate, keep_unused=True)
    sh = NamedSharding(mesh, PartitionSpec("core"))
    zeros = jax.jit(
        lambda: tuple(jnp.zeros((n_cores * a.shape[0], *a.shape[1:]), a.dtype)
                      for a in out_avals),
        out_shardings=(sh,) * len(out_avals))()
    concat_in = [np.concatenate([np.asarray(m[k]) for m in in_maps], axis=0)
                 for k in in_names]
    outs = [np.asarray(o) for o in sharded(*concat_in, *zeros)]
    per_core = []
    for c in range(n_cores):
        d = {}
        for name, arr in zip(out_names, outs):
            s0 = arr.shape[0] // n_cores
            d[name] = arr[c * s0:(c + 1) * s0]
        per_core.append(d)
    return per_core


def kernel(x, w_q, w_k, w_v, freqs_cos, freqs_sin, _want_results=False, _trace=False):
    if "nc" not in _CACHE:
        _CACHE["nc"] = _build()
    nc = _CACHE["nc"]
    in_maps = _prep_inputs(np.asarray(x, np.float32), np.asarray(w_q, np.float32),
                           np.asarray(w_k, np.float32), np.asarray(w_v, np.float32),
                           np.asarray(freqs_cos, np.float32),
                           np.asarray(freqs_sin, np.float32))
    if _trace:
        kr = run_bass_kernel_spmd(nc, in_maps, core_ids=list(range(8)), trace=True)
        out = _assemble(kr.results)
        return (out, kr) if _want_results else out
    try:
        results = _run_pjrt(nc, in_maps)
    except Exception as e:
        print(f"kernel: _run_pjrt failed ({type(e).__name__}: {e}); "
              "falling back to run_bass_kernel_spmd", file=sys.stderr)
        kr = run_bass_kernel_spmd(nc, in_maps, core_ids=list(range(8)))
        results = kr.results
    out = _assemble(results)
    if _want_results:
        return out, results
    return out
